# revision 9
# baseline (speedup 1.0000x reference)
"""Trainium2 Bass kernel for nn_L1OutUB (L1-out upper bound contrastive loss).

Math: the reference builds a [B,B,B] tensor `inpt[a,i,j] = all_probs[i,j] +
(-20 if a==i else 0)` and logsumexps over `a`.  That logsumexp is exactly
`all_probs[i,j] + log(B-1+e^-20)`, so

    result = mean(positive) - mean(all_probs) - log1p(e^-20 / (B-1))

`sum_j all_probs[i,j]` collapses onto per-column moments of y, and the
-0.5*logvar terms cancel between positive and negative.  Expanding the
positive-branch square as well, every remaining term is a contraction of
per-core row sums:

    A[d]  = sum_i iv[i,d]          C[d]  = sum_i mu[i,d] iv[i,d]
    D     = sum_{i,d} mu^2 iv      U1    = sum_{i,d} mu y iv   (matched y)
    U2    = sum_{i,d} y^2 iv       S2[d] = sum_j y[j,d]^2      M1[d] = sum_j y[j,d]

    P     = D - 2 U1 + U2          (positive-branch quadratic)
    result = -(P/2B) + (1/2B^2)(S2.A - 2 M1.C + B D) - log1p(e^-20/(B-1))

Sharding: rows of x/y across 8 cores (64 rows each); each core emits its
partial vectors [128, 7] = (A, C, Dv, U1v, U2v, S2, M1); the host sums the
8 partials and does three dot products (the "all-reduce").

Device-side structure per core (layout [d=128 partitions, r=64 free]):
  - host pre-transposes x (xT chunks) and y (yT), packs both MLPs' layer-1
    weights into one dense f32r [128,18] lhsT per chunk whose two zero
    columns, via relu(0 + bias=1), manufacture ones-rows that fold the
    layer-2 biases into the matmuls.  float32r keeps each matmul a single
    PE pass (fp32 runs as two) at near-fp32 precision.
  - PE: 6 accumulating L1 matmuls -> relu -> 2 L2 matmuls whose lhsT are
    zero-padded to base partition 0 ([18,128] each); z's matmul goes first
    so tanh starts while mu's matmul still runs.
  - Scalar: relu, tanh(scale=-1), exp (iv = exp(-tanh(z)); logvar cancels).
  - DVE: iv + 4 products live in one [128, 5*64] tile; a single segmented
    tensor_reduce emits A, C, Dv, U1v, U2v at once.  y moments done early.
  - No transposes, no GpSimd, no collectives; one [128,7] output DMA.
"""

import numpy as np

import concourse.bacc as bacc
import concourse.tile as tile
from concourse import mybir

F32 = mybir.dt.float32
F32R = mybir.dt.float32r
AF = mybir.ActivationFunctionType
ALU = mybir.AluOpType
AX = mybir.AxisListType

B, X_DIM, Y_DIM, HID = 512, 768, 128, 8
N_CORES = 8
R = B // N_CORES          # rows per core = 64
XC = X_DIM // 128         # x feature chunks = 6
WCOL = 18                 # packed L1 lhsT cols: 0:8 mu, 8 zero, 9:17 lv, 17 zero
CHUNK = WCOL + R          # per-chunk blob cols = 82

_CACHE = {}


def _build():
    nc = bacc.Bacc("TRN2", target_bir_lowering=False, debug=False,
                   num_devices=1)

    # a1/a2: 3 chunks each of [w1p_k [128,18] | xT_k [128,64]], bf16
    a1_d = nc.dram_tensor("a1", [128, 3 * CHUNK], F32R, kind="ExternalInput")
    a2_d = nc.dram_tensor("a2", [128, 3 * CHUNK], F32R, kind="ExternalInput")
    # yb: cols 0:64 = yT (this core's y slice, transposed), col 64 = b1vec
    #     (rows 0:8 b1_mu, row 8 = 1.0, rows 9:17 b1_lv, row 17 = 1.0)
    yb_d = nc.dram_tensor("yb", [128, 65], F32, kind="ExternalInput")
    # w2: cols 0:128 = mu block (rows 0:8 w2_mu, row 8 b2_mu, rows 9:18 zero)
    #     cols 128:256 = lv block (rows 0:9 zero, rows 9:17 w2_lv, row 17 b2_lv)
    w2_d = nc.dram_tensor("w2", [WCOL, 256], F32R, kind="ExternalInput")
    # out columns: A, C, Dv, U1v, U2v, S2, M1
    out_d = nc.dram_tensor("out", [128, 7], F32, kind="ExternalOutput")

    with tile.TileContext(nc) as tc:
        with (
            tc.tile_pool(name="sb", bufs=1) as sb,
            tc.tile_pool(name="ps", bufs=1, space="PSUM") as ps,
        ):
            # ---- loads: x-bearing blobs first on both rings ----
            a1_s = sb.tile([128, 3 * CHUNK], F32R, tag="a1")
            nc.sync.dma_start(out=a1_s[:], in_=a1_d[:])
            a2_s = sb.tile([128, 3 * CHUNK], F32R, tag="a2")
            nc.scalar.dma_start(out=a2_s[:], in_=a2_d[:])
            yb_s = sb.tile([128, 65], F32, tag="yb")
            nc.sync.dma_start(out=yb_s[:], in_=yb_d[:])
            w2_s = sb.tile([WCOL, 256], F32R, tag="w2")
            nc.scalar.dma_start(out=w2_s[:], in_=w2_d[:])

            yT = yb_s[:, 0:R]
            outv = sb.tile([128, 7], F32, tag="outv")
            ysq_s = sb.tile([128, R], F32, tag="ysq")

            # ---- early y moments (only need this core's slice) ----
            nc.vector.tensor_mul(ysq_s[:], yT, yT)
            nc.vector.tensor_reduce(out=outv[:, 5:6], in_=ysq_s[:],
                                    axis=AX.X, op=ALU.add)
            nc.vector.tensor_reduce(out=outv[:, 6:7], in_=yT,
                                    axis=AX.X, op=ALU.add)

            # ---- L1 (both nets fused): ps1 = w1p.T @ xT over 6 chunks ----
            ps1 = ps.tile([WCOL, R], F32, tag="ps1")
            for k in range(XC):
                src = a1_s if k < 3 else a2_s
                j = (k % 3) * CHUNK
                nc.tensor.matmul(ps1[:], src[:, j:j + WCOL],
                                 src[:, j + WCOL:j + CHUNK],
                                 start=(k == 0), stop=(k == XC - 1))
            hb_s = sb.tile([WCOL, R], F32R, tag="hb")
            nc.scalar.activation(out=hb_s[:], in_=ps1[:], func=AF.Relu,
                                 bias=yb_s[0:WCOL, 64:65])

            # ---- L2: z first (tanh is the long pole), then mu ----
            ps2l = ps.tile([128, R], F32, tag="ps2l")
            ps2m = ps.tile([128, R], F32, tag="ps2m")
            nc.tensor.matmul(ps2l[:], w2_s[:, 128:256], hb_s[:],
                             start=True, stop=True)
            nc.tensor.matmul(ps2m[:], w2_s[:, 0:128], hb_s[:],
                             start=True, stop=True)
            mu = ps2m[:]

            # ---- iv = exp(-tanh(z)) straight into the products tile ----
            prods = sb.tile([128, 5, R], F32, tag="prods")
            iv = prods[:, 0, :]
            th_s = sb.tile([128, R], F32, tag="th")
            nc.scalar.activation(out=th_s[:], in_=ps2l[:],
                                 func=AF.Tanh, scale=-1.0)
            nc.scalar.activation(out=iv, in_=th_s[:], func=AF.Exp)

            # ---- products on DVE; one segmented reduce -> A,C,Dv,U1v,U2v ----
            t1 = prods[:, 1, :]
            nc.vector.tensor_mul(t1, mu, iv)
            nc.vector.tensor_mul(prods[:, 2, :], t1, mu)
            nc.vector.tensor_mul(prods[:, 3, :], t1, yT)
            nc.vector.tensor_mul(prods[:, 4, :], ysq_s[:], iv)
            nc.vector.tensor_reduce(out=outv[:, 0:5], in_=prods[:],
                                    axis=AX.X, op=ALU.add)

            nc.sync.dma_start(out=out_d[:], in_=outv[:])

    nc.compile()
    return nc


def _get_nc():
    if "nc" not in _CACHE:
        _CACHE["nc"] = _build()
    return _CACHE["nc"]


def _pack(x_samples, y_samples, w1_mu, b1_mu, w2_mu, b2_mu,
          w1_lv, b1_lv, w2_lv, b2_lv):
    f = np.float32
    w1m = np.asarray(w1_mu, f).reshape(XC, 128, HID)
    w1l = np.asarray(w1_lv, f).reshape(XC, 128, HID)
    w1p = np.zeros((XC, 128, WCOL), f)
    w1p[:, :, 0:8] = w1m
    w1p[:, :, 9:17] = w1l

    b1vec = np.zeros((128, 1), f)
    b1vec[0:8, 0] = np.asarray(b1_mu, f)
    b1vec[8, 0] = 1.0
    b1vec[9:17, 0] = np.asarray(b1_lv, f)
    b1vec[17, 0] = 1.0

    w2 = np.zeros((WCOL, 256), f)
    w2[0:8, 0:128] = np.asarray(w2_mu, f)
    w2[8, 0:128] = np.asarray(b2_mu, f)
    w2[9:17, 128:256] = np.asarray(w2_lv, f)
    w2[17, 128:256] = np.asarray(b2_lv, f)

    x = np.asarray(x_samples, f)
    y = np.asarray(y_samples, f)
    in_maps = []
    for c in range(N_CORES):
        xs = x[c * R:(c + 1) * R]                       # [64, 768]
        xT = xs.reshape(R, XC, 128).transpose(1, 2, 0)  # [6, 128, 64]
        chunks = np.empty((XC, 128, CHUNK), f)
        chunks[:, :, 0:WCOL] = w1p
        chunks[:, :, WCOL:CHUNK] = xT
        a1 = np.ascontiguousarray(
            chunks[0:3].transpose(1, 0, 2).reshape(128, 3 * CHUNK))
        a2 = np.ascontiguousarray(
            chunks[3:6].transpose(1, 0, 2).reshape(128, 3 * CHUNK))
        yb = np.empty((128, 65), f)
        yb[:, 0:R] = y[c * R:(c + 1) * R].T
        yb[:, 64:65] = b1vec
        in_maps.append({"a1": a1, "a2": a2, "yb": yb, "w2": w2})
    return in_maps


def kernel(x_samples, y_samples, w1_mu, b1_mu, w2_mu, b2_mu,
           w1_lv, b1_lv, w2_lv, b2_lv, **profile_kwargs):
    from concourse import bass_utils

    in_maps = _pack(x_samples, y_samples, w1_mu, b1_mu, w2_mu, b2_mu,
                    w1_lv, b1_lv, w2_lv, b2_lv)
    nc = _get_nc()
    res = bass_utils.run_bass_kernel_spmd(
        nc, in_maps, core_ids=list(range(N_CORES)), **profile_kwargs
    )
    acc = np.zeros((128, 7), np.float64)
    for m in res.results:
        acc += m["out"].astype(np.float64)
    A, C, Dv, U1v, U2v, S2, M1 = (acc[:, j] for j in range(7))
    D = Dv.sum()
    P = D - 2.0 * U1v.sum() + U2v.sum()
    neg = (S2 @ A - 2.0 * (M1 @ C) + B * D) / (2.0 * B * B)
    total = -P / (2.0 * B) + neg - np.log1p(np.exp(-20.0) / (B - 1.0))
    out = np.array(total, dtype=np.float32)
    if profile_kwargs:
        return out, res
    return out
